# revision 8
# baseline (speedup 1.0000x reference)
"""LoRA self-attention TRN2 kernel (8 NeuronCores, SPMD) — v2.

Sharding: core c = (b, hp) with b = c // 4 (batch), hp = c % 4 (head group of
4 heads = 256 channels). Each core computes q/k/v projections (+LoRA) for its
256 output channels from the full x[b], runs attention for its 4 heads, and a
partial output projection over its 256 context channels. Host sums the 4
partials per batch element and adds bo.

Numerics: q/k projections and the [k,q]-oriented QK^T scores use 3-term bf16
hi/lo splits (fp32-grade); the softmax shift m-hat comes from a single-bf16
[q,k] score pass (max error ~±65 ≪ the ±80 exp-safety slack, and the shift
cancels exactly in softmax). P·V and the output projection run in bf16.

Structure (per head h, q-block qb of 512):
  1. m-hat pass: [q,k] scores (1 matmul/tile) -> DVE row-max -> x0.125 ->
     DRAM bounce -> one DMA that transposes [128,4]->[1,512] AND broadcasts
     to [128,512].
  2. sT pass: scores in [k,q] (3 split matmuls per k-tile) -> fused DVE
     scalar_tensor_tensor (x0.125 - m-hat) -> bf16 -> ACT exp -> pT tiles.
     No PE transposes of P anywhere.
  3. PV: ctxT[d,q] += v_aug[k,d+ones].T @ pT[k,q]; the ones column makes
     row 64 the softmax normalizer Z. Normalize during PSUM evacuation with
     reciprocal + gpsimd partition_broadcast.
"""
import sys

sys.path.insert(0, "/opt/trn_rl_repo")

from contextlib import ExitStack

import numpy as np
import ml_dtypes

import concourse.bass as bass
import concourse.tile as tile
from concourse import bacc, mybir
from concourse.bass import ts
from concourse.bass_utils import run_bass_kernel_spmd

F32 = mybir.dt.float32
BF16 = mybir.dt.bfloat16
bf16 = ml_dtypes.bfloat16
AX = mybir.AxisListType
Exp = mybir.ActivationFunctionType.Exp
MULT = mybir.AluOpType.mult
SUB = mybir.AluOpType.subtract

T = 2048          # sequence length
E = 1024          # embed
OL = 256          # local output channels (4 heads)
D = 64            # head dim
NH = 4            # local heads
R = 8             # lora rank
CI = 8            # contraction chunks of 128 over E
NS = 4            # 512-wide slices over T
TC = 16           # 128-wide tiles over T
VW = 65           # v-aug width per head (64 + ones column)

_CACHE = {}


def _build():
    if "nc" in _CACHE:
        return _CACHE["nc"]

    nc = bacc.Bacc("TRN2", target_bir_lowering=False, debug=False)

    # ---- DRAM I/O ----
    xth_d = nc.dram_tensor("xth", [E, T], BF16, kind="ExternalInput")
    xtl_d = nc.dram_tensor("xtl", [E, T], BF16, kind="ExternalInput")
    w_d = {}
    for p in "qkv":
        for s in "hl":
            w_d[p + s] = nc.dram_tensor(f"w{p}{s}", [E, OL], BF16, kind="ExternalInput")
    woT_d = nc.dram_tensor("woT", [OL, E], BF16, kind="ExternalInput")
    ah_d = nc.dram_tensor("ah", [E, 3 * R], BF16, kind="ExternalInput")
    al_d = nc.dram_tensor("al", [E, 3 * R], BF16, kind="ExternalInput")
    b_d = {}
    for p in "qkv":
        for s in "hl":
            b_d[p + s] = nc.dram_tensor(f"b{p}{s}", [R, OL], BF16, kind="ExternalInput")
    ident_d = nc.dram_tensor("ident", [128, 128], BF16, kind="ExternalInput")
    outp_d = nc.dram_tensor("outp", [T, E], F32, kind="ExternalOutput")

    with tile.TileContext(nc) as tc, ExitStack() as ctx:
        # ---------------- persistent pool ----------------
        pers = ctx.enter_context(tc.tile_pool(name="pers", bufs=1))
        qth_t = [pers.tile([128, T], BF16, tag=f"qth{c}", name=f"qth{c}") for c in range(2)]
        qtl_t = [pers.tile([128, T], BF16, tag=f"qtl{c}", name=f"qtl{c}") for c in range(2)]
        kth_t = [pers.tile([128, T], BF16, tag=f"kth{c}", name=f"kth{c}") for c in range(2)]
        ktl_t = [pers.tile([128, T], BF16, tag=f"ktl{c}", name=f"ktl{c}") for c in range(2)]
        v16 = [pers.tile([128, NH * VW], BF16, tag=f"v16_{i}", name=f"v16_{i}")
               for i in range(TC)]
        ctxT_t = [pers.tile([128, T], BF16, tag=f"ctxT{c}", name=f"ctxT{c}") for c in range(2)]

        # ---------------- phase 1: projections ----------------
        with ExitStack() as ph1:
            ld = ph1.enter_context(tc.tile_pool(name="ld", bufs=1))
            wpool = ph1.enter_context(tc.tile_pool(name="wpool", bufs=2))
            pps = ph1.enter_context(tc.tile_pool(name="pps", bufs=2, space="PSUM"))
            upsp = ph1.enter_context(tc.tile_pool(name="upsp", bufs=1, space="PSUM"))
            vtrp = ph1.enter_context(tc.tile_pool(name="vtrp", bufs=2, space="PSUM"))

            ident = ld.tile([128, 128], BF16, name="ident")
            nc.sync.dma_start(out=ident, in_=ident_d[:, :])

            xth_t, xtl_t, ah_t, al_t = [], [], [], []
            for ci in range(CI):
                t_ = ld.tile([128, T], BF16, tag=f"xth{ci}", name=f"xth{ci}")
                nc.sync.dma_start(out=t_, in_=xth_d[ts(ci, 128), :])
                xth_t.append(t_)
                t_ = ld.tile([128, T], BF16, tag=f"xtl{ci}", name=f"xtl{ci}")
                nc.sync.dma_start(out=t_, in_=xtl_d[ts(ci, 128), :])
                xtl_t.append(t_)
                t_ = ld.tile([128, 3 * R], BF16, tag=f"ah{ci}", name=f"ah{ci}")
                nc.sync.dma_start(out=t_, in_=ah_d[ts(ci, 128), :])
                ah_t.append(t_)
                t_ = ld.tile([128, 3 * R], BF16, tag=f"al{ci}", name=f"al{ci}")
                nc.sync.dma_start(out=t_, in_=al_d[ts(ci, 128), :])
                al_t.append(t_)
            b_t = {}
            for key, d in b_d.items():
                t_ = ld.tile([R, OL], BF16, tag=f"b{key}", name=f"b{key}")
                nc.sync.dma_start(out=t_, in_=d[:, :])
                b_t[key] = t_

            # --- u_all = x @ A_all (split3), shared M=24 pass ---
            ups = upsp.tile([3 * R, T], F32, name="ups")
            for ns in range(NS):
                sl = ts(ns, 512)
                n_mm = 3 * CI
                i = 0
                for ci in range(CI):
                    for a_t, x_t in ((ah_t[ci], xth_t[ci]), (ah_t[ci], xtl_t[ci]),
                                     (al_t[ci], xth_t[ci])):
                        nc.tensor.matmul(ups[:, sl], a_t, x_t[:, sl],
                                         start=(i == 0), stop=(i == n_mm - 1))
                        i += 1
            uf = ld.tile([3 * R, T], F32, name="uf")
            nc.any.tensor_copy(uf, ups)
            u_bf = {}
            for pi, p in enumerate("qkv"):
                upf = ld.tile([R, T], F32, name=f"u{p}f")
                nc.sync.dma_start(out=upf, in_=uf[pi * R:(pi + 1) * R, :])
                uh = ld.tile([R, T], BF16, name=f"u{p}h")
                ul = ld.tile([R, T], BF16, name=f"u{p}l")
                nc.vector.tensor_copy(uh, upf)
                nc.vector.tensor_sub(ul, upf, uh)
                u_bf[p + "h"], u_bf[p + "l"] = uh, ul

            # --- projections, transposed layout [OL, T] ---
            for p, outs in (("q", (qth_t, qtl_t)), ("k", (kth_t, ktl_t)), ("v", None)):
                wh_t, wl_t = [], []
                for ci in range(CI):
                    t_ = wpool.tile([128, OL], BF16, tag=f"wh{ci}", name=f"wh{ci}")
                    nc.sync.dma_start(out=t_, in_=w_d[p + "h"][ts(ci, 128), :])
                    wh_t.append(t_)
                    if p != "v":
                        t_ = wpool.tile([128, OL], BF16, tag=f"wl{ci}", name=f"wl{ci}")
                        nc.sync.dma_start(out=t_, in_=w_d[p + "l"][ts(ci, 128), :])
                        wl_t.append(t_)
                vth_t = None
                if p == "v":
                    vth_t = [wpool.tile([128, T], BF16, tag=f"vth{c}", name=f"vth{c}",
                                        bufs=1) for c in range(2)]
                for oc in range(2):
                    osl = ts(oc, 128)
                    for ns in range(NS):
                        sl = ts(ns, 512)
                        ps = pps.tile([128, 512], F32, tag="proj", name="proj")
                        if p == "v":
                            base = [(wh_t[ci], xth_t[ci]) for ci in range(CI)]
                        else:
                            base = []
                            for ci in range(CI):
                                base += [(wh_t[ci], xth_t[ci]), (wh_t[ci], xtl_t[ci]),
                                         (wl_t[ci], xth_t[ci])]
                        lora = [(b_t[p + "h"], u_bf[p + "h"]),
                                (b_t[p + "h"], u_bf[p + "l"]),
                                (b_t[p + "l"], u_bf[p + "h"])]
                        n_mm = len(base) + len(lora)
                        for i, (a, b_) in enumerate(base):
                            nc.tensor.matmul(ps, a[:, osl], b_[:, sl],
                                             start=(i == 0), stop=False)
                        for j, (bb, uu) in enumerate(lora):
                            nc.tensor.matmul(ps, bb[:, osl], uu[:, sl], start=False,
                                             stop=(len(base) + j == n_mm - 1))
                        if p == "v":
                            nc.any.tensor_copy(vth_t[oc][:, sl], ps)
                        else:
                            hi, lo = outs
                            nc.any.tensor_copy(hi[oc][:, sl], ps)
                            nc.vector.tensor_sub(lo[oc][:, sl], ps, hi[oc][:, sl])
                if p == "v":
                    # v16 tiles: per head 64 v-cols + a ones column (Z trick)
                    for tci in range(TC):
                        nc.vector.memset(v16[tci], 1.0)
                    for oc in range(2):
                        for tci in range(TC):
                            tp = vtrp.tile([128, 128], BF16, tag="vtr", name="vtr")
                            nc.tensor.transpose(tp, vth_t[oc][:, ts(tci, 128)], ident)
                            h0, h1 = 2 * oc, 2 * oc + 1
                            nc.any.tensor_copy(v16[tci][:, h0 * VW:h0 * VW + 64],
                                               tp[:, 0:64])
                            nc.any.tensor_copy(v16[tci][:, h1 * VW:h1 * VW + 64],
                                               tp[:, 64:128])

        # ---------------- phase 3: attention ----------------
        with ExitStack() as ph3:
            att = ph3.enter_context(tc.tile_pool(name="att", bufs=2))
            ptp = ph3.enter_context(tc.tile_pool(name="ptp", bufs=2))
            scr = ph3.enter_context(tc.tile_pool(name="scr", bufs=4))
            drp = ph3.enter_context(tc.tile_pool(name="drp", bufs=2, space="DRAM"))
            msp = ph3.enter_context(tc.tile_pool(name="msp", bufs=1, space="PSUM"))
            sps = ph3.enter_context(tc.tile_pool(name="sps", bufs=3, space="PSUM"))
            cps = ph3.enter_context(tc.tile_pool(name="cps", bufs=1, space="PSUM"))

            ones_row = att.tile([1, 128], BF16, name="ones_row", bufs=1)
            nc.vector.memset(ones_row, 1.0)

            for h in range(NH):
                ch, hh = h // 2, h % 2
                pr = hh * 64
                qh_c, ql_c = qth_t[ch], qtl_t[ch]
                kh_c, kl_c = kth_t[ch], ktl_t[ch]
                for qb in range(NS):
                    qsl = ts(qb, 512)
                    # --- m-hat pass: [q,k] single-bf16 scores, row max ---
                    rm4 = att.tile([128, 4], F32, name="rm4")
                    for ql_i in range(4):
                        qt = qb * 4 + ql_i
                        ms = msp.tile([128, T], F32, tag="ms", name="ms")
                        for ns in range(NS):
                            nc.tensor.matmul(ms[:, ts(ns, 512)],
                                             qh_c[pr:pr + 64, ts(qt, 128)],
                                             kh_c[pr:pr + 64, ts(ns, 512)],
                                             start=True, stop=True,
                                             tile_position=(pr, 0))
                        nc.vector.reduce_max(out=rm4[:, ql_i:ql_i + 1], in_=ms,
                                             axis=AX.X)
                    rm4s = att.tile([128, 4], BF16, name="rm4s")
                    nc.vector.tensor_scalar_mul(rm4s, rm4, -1.0)
                    dr = drp.tile([128, 4], BF16, name="mh_dr")
                    nc.sync.dma_start(out=dr, in_=rm4s)
                    # transpose via DRAM bounce: mh_row[0, ql_i*128 + q] = dr[q, ql_i]
                    mh_row = att.tile([1, 512], BF16, name="mh_row")
                    src = bass.AP(tensor=dr.tensor, offset=dr.offset,
                                  ap=[[1, 4], [4, 128]])
                    nc.sync.dma_start(out=mh_row, in_=src)

                    # --- sT pass: [k,q] split3 scores, minus m-hat via a K=1
                    # rank-1 accumulating matmul (ones^T @ -m-hat), then exp
                    # straight out of PSUM with the 1/8 scale ---
                    pT = [ptp.tile([128, 512], BF16, tag=f"pt{i}", name=f"pt{i}")
                          for i in range(TC)]
                    for kt in range(TC):
                        st = sps.tile([128, 512], F32, tag="st", name="st")
                        for i, (a, b_) in enumerate((
                                (kh_c[pr:pr + 64, ts(kt, 128)], qh_c[pr:pr + 64, qsl]),
                                (kh_c[pr:pr + 64, ts(kt, 128)], ql_c[pr:pr + 64, qsl]),
                                (kl_c[pr:pr + 64, ts(kt, 128)], qh_c[pr:pr + 64, qsl]))):
                            nc.tensor.matmul(st, a, b_, start=(i == 0), stop=False,
                                             tile_position=(pr, 0))
                        nc.tensor.matmul(st, ones_row, mh_row, start=False, stop=True,
                                         tile_position=(0, 0))
                        nc.scalar.activation(out=pT[kt], in_=st, func=Exp, scale=0.125)
                    # --- PV with ones column ---
                    cxa = cps.tile([VW, 512], F32, tag="cxa", name="cxa")
                    for kt in range(TC):
                        nc.tensor.matmul(cxa, v16[kt][:, h * VW:(h + 1) * VW], pT[kt],
                                         start=(kt == 0), stop=(kt == TC - 1))
                    # --- normalize by Z (row 64) during evacuation ---
                    zrow = att.tile([1, 512], F32, name="zrow")
                    nc.vector.tensor_copy(zrow, cxa[64:65, :])
                    z_bc = att.tile([64, 512], F32, name="z_bc")
                    nc.gpsimd.partition_broadcast(z_bc, zrow, channels=64)
                    rcp_bc = att.tile([64, 512], F32, name="rcp_bc")
                    nc.vector.reciprocal(rcp_bc, z_bc)
                    nc.vector.tensor_mul(ctxT_t[ch][pr:pr + 64, qsl], cxa[0:64, :],
                                         rcp_bc)

        # ---------------- phase 4: output projection ----------------
        with ExitStack() as ph4:
            ost_p = ph4.enter_context(tc.tile_pool(name="ost", bufs=3))
            ops = ph4.enter_context(tc.tile_pool(name="ops", bufs=2, space="PSUM"))
            woT_t = []
            for cc in range(2):
                t_ = ost_p.tile([128, E], BF16, tag=f"woT{cc}", name=f"woT{cc}")
                nc.sync.dma_start(out=t_, in_=woT_d[ts(cc, 128), :])
                woT_t.append(t_)
            for tci in range(TC):
                tsl = ts(tci, 128)
                ost = ost_p.tile([128, E], F32, tag="ost", name="ost")
                for no in range(2):
                    op_t = ops.tile([128, 512], F32, tag=f"op{no}", name=f"op{no}")
                    for cc in range(2):
                        nc.tensor.matmul(op_t, ctxT_t[cc][:, tsl],
                                         woT_t[cc][:, ts(no, 512)],
                                         start=(cc == 0), stop=(cc == 1))
                    nc.any.tensor_copy(ost[:, ts(no, 512)], op_t)
                nc.sync.dma_start(out=outp_d[tsl, :], in_=ost)

    nc.compile()
    _CACHE["nc"] = nc
    return nc


def _split(a):
    h = a.astype(bf16)
    l = (a - h.astype(np.float32)).astype(bf16)
    return h, l


def _shard(inputs):
    x = np.asarray(inputs["x"], np.float32)
    Wo = np.asarray(inputs["Wo"], np.float32)
    A_all = np.concatenate([np.asarray(inputs["Aq"], np.float32),
                            np.asarray(inputs["Ak"], np.float32),
                            np.asarray(inputs["Av"], np.float32)], axis=1)
    ident = np.eye(128, dtype=np.float32).astype(bf16)
    in_maps = []
    for core in range(8):
        b, hp = core // 4, core % 4
        o0 = hp * OL
        xT = np.ascontiguousarray(x[b].T)
        xh, xl = _split(xT)
        m = {"xth": xh, "xtl": xl, "ident": ident}
        for p in "qkv":
            W = np.asarray(inputs["W" + p], np.float32)
            Ws = np.ascontiguousarray(W[o0:o0 + OL, :].T)
            m["w%sh" % p], m["w%sl" % p] = _split(Ws)
            B = np.asarray(inputs["B" + p], np.float32)[:, o0:o0 + OL] * 2.0
            m["b%sh" % p], m["b%sl" % p] = _split(B)
        m["woT"] = np.ascontiguousarray(Wo[:, o0:o0 + OL].T).astype(bf16)
        m["ah"], m["al"] = _split(A_all)
        in_maps.append(m)
    return in_maps


def _run(inputs, trace=False, **kw):
    nc = _build()
    in_maps = _shard(inputs)
    res = run_bass_kernel_spmd(nc, in_maps, core_ids=list(range(8)), trace=trace, **kw)
    bo = np.asarray(inputs["bo"], np.float32)
    parts = [res.results[c]["outp"].astype(np.float64) for c in range(8)]
    out = np.stack([sum(parts[0:4]), sum(parts[4:8])]) + bo.astype(np.float64)
    return out.astype(np.float32), res


def kernel(**inputs):
    out, _ = _run(inputs)
    return out


# revision 9
# speedup vs baseline: 1.4469x; 1.4469x over previous
"""LoRA self-attention TRN2 kernel (8 NeuronCores, SPMD) — v4.

Sharding: core c = (b, hp) with b = c // 4 (batch), hp = c % 4 (head group of
4 heads = 256 channels). Each core computes q/k/v projections (+LoRA) for its
256 output channels from the full x[b], runs attention for its 4 heads, and a
partial output projection over its 256 context channels. Host sums the 4
partials per batch element and adds bo.

Numerics: q/k projections and the [k,q]-oriented QK^T scores use bf16 hi/lo
splits (s = kh·qh + kl·qh + kh·ql, fp32-grade); the softmax shift m-hat comes
from a single-bf16 [q,k] score pass (error ≪ the exp-safety slack; the shift
cancels exactly in softmax). P·V and the output projection run in bf16.

PE-row reduction tricks (the chip power-caps sustained 8-core PE activity, so
wall time tracks streamed matmul rows):
  - K-stacked split: per head, K tiles hold [k_hi; k_lo] on 128 partitions and
    q_hi is duplicated on both halves, so kh·qh + kl·qh is ONE K=128 matmul;
    only kh·ql needs a second K=64 matmul.
  - ones-column on V makes PV row 64 the softmax normalizer Z (no reduce).
  - m-hat subtraction is fused into the PSUM evacuation on DVE
    (scalar_tensor_tensor), exp runs on ACT from SBUF bf16.
  - when every LoRA B factor is zero (standard LoRA init), a specialized
    no-LoRA program is compiled and used; the general path handles B != 0.
"""
import sys

sys.path.insert(0, "/opt/trn_rl_repo")

from contextlib import ExitStack

import numpy as np
import ml_dtypes

import concourse.bass as bass
import concourse.tile as tile
from concourse import bacc, mybir
from concourse.bass import ts
from concourse.bass_utils import run_bass_kernel_spmd

F32 = mybir.dt.float32
BF16 = mybir.dt.bfloat16
bf16 = ml_dtypes.bfloat16
AX = mybir.AxisListType
Exp = mybir.ActivationFunctionType.Exp
MULT = mybir.AluOpType.mult
SUB = mybir.AluOpType.subtract

T = 2048          # sequence length
E = 1024          # embed
OL = 256          # local output channels (4 heads)
D = 64            # head dim
NH = 4            # local heads
R = 8             # lora rank
CI = 8            # contraction chunks of 128 over E
NS = 4            # 512-wide slices over T
TC = 16           # 128-wide tiles over T
VW = 65           # v-aug width per head (64 + ones column)

_CACHE = {}


def _build(lora=True):
    key = ("nc", lora)
    if key in _CACHE:
        return _CACHE[key]

    nc = bacc.Bacc("TRN2", target_bir_lowering=False, debug=False)

    # ---- DRAM I/O ----
    xth_d = nc.dram_tensor("xth", [E, T], BF16, kind="ExternalInput")
    xtl_d = nc.dram_tensor("xtl", [E, T], BF16, kind="ExternalInput")
    w_d = {}
    for p in "qkv":
        for s in "hl":
            w_d[p + s] = nc.dram_tensor(f"w{p}{s}", [E, OL], BF16, kind="ExternalInput")
    woT_d = nc.dram_tensor("woT", [OL, E], BF16, kind="ExternalInput")
    if lora:
        ah_d = nc.dram_tensor("ah", [E, 3 * R], BF16, kind="ExternalInput")
        al_d = nc.dram_tensor("al", [E, 3 * R], BF16, kind="ExternalInput")
        b_d = {}
        for p in "qkv":
            for s in "hl":
                b_d[p + s] = nc.dram_tensor(f"b{p}{s}", [R, OL], BF16,
                                            kind="ExternalInput")
    ident_d = nc.dram_tensor("ident", [128, 128], BF16, kind="ExternalInput")
    outp_d = nc.dram_tensor("outp", [T, E], F32, kind="ExternalOutput")

    with tile.TileContext(nc) as tc, ExitStack() as ctx:
        # ---------------- persistent tiles ----------------
        # Per-head score operand layouts:
        #   khl[h] [128,T]: rows 0:64 = kT_hi(h), rows 64:128 = kT_lo(h)
        #   qhh[h] [128,T]: rows 0:64 = qT_hi(h), rows 64:128 = qT_hi(h) (dup)
        #   qlo[h] [64,T]:  qT_lo(h)
        pers = ctx.enter_context(tc.tile_pool(name="pers", bufs=1))
        khl = [pers.tile([128, T], BF16, name=f"khl{h}") for h in range(NH)]
        qhh = [pers.tile([128, T], BF16, name=f"qhh{h}") for h in range(NH)]
        qlo = [pers.tile([64, T], BF16, name=f"qlo{h}") for h in range(NH)]
        v16 = [pers.tile([128, NH * VW], BF16, name=f"v16_{i}") for i in range(TC)]
        ctxT_t = [pers.tile([128, T], BF16, name=f"ctxT{c}") for c in range(2)]

        # ---------------- phase 1: projections ----------------
        with ExitStack() as ph1:
            ld = ph1.enter_context(tc.tile_pool(name="ld", bufs=1))
            wpool = ph1.enter_context(tc.tile_pool(name="wpool", bufs=2))
            pps = ph1.enter_context(tc.tile_pool(name="pps", bufs=2, space="PSUM"))
            upsp = ph1.enter_context(tc.tile_pool(name="upsp", bufs=1, space="PSUM"))
            vtrp = ph1.enter_context(tc.tile_pool(name="vtrp", bufs=2, space="PSUM"))

            ident = ld.tile([128, 128], BF16, name="ident")
            nc.sync.dma_start(out=ident, in_=ident_d[:, :])

            xth_t, xtl_t, ah_t, al_t = [], [], [], []
            for ci in range(CI):
                t_ = ld.tile([128, T], BF16, name=f"xth{ci}")
                nc.sync.dma_start(out=t_, in_=xth_d[ts(ci, 128), :])
                xth_t.append(t_)
                t_ = ld.tile([128, T], BF16, name=f"xtl{ci}")
                nc.sync.dma_start(out=t_, in_=xtl_d[ts(ci, 128), :])
                xtl_t.append(t_)
                if lora:
                    t_ = ld.tile([128, 3 * R], BF16, name=f"ah{ci}")
                    nc.sync.dma_start(out=t_, in_=ah_d[ts(ci, 128), :])
                    ah_t.append(t_)
                    t_ = ld.tile([128, 3 * R], BF16, name=f"al{ci}")
                    nc.sync.dma_start(out=t_, in_=al_d[ts(ci, 128), :])
                    al_t.append(t_)
            u_bf = {}
            b_t = {}
            if lora:
                for key2, d in b_d.items():
                    t_ = ld.tile([R, OL], BF16, name=f"b{key2}")
                    nc.sync.dma_start(out=t_, in_=d[:, :])
                    b_t[key2] = t_

                # u_all = x @ A_all (split3), shared M=24 pass
                ups = upsp.tile([3 * R, T], F32, name="ups")
                for ns in range(NS):
                    sl = ts(ns, 512)
                    n_mm = 3 * CI
                    i = 0
                    for ci in range(CI):
                        for a_t, x_t in ((ah_t[ci], xth_t[ci]), (ah_t[ci], xtl_t[ci]),
                                         (al_t[ci], xth_t[ci])):
                            nc.tensor.matmul(ups[:, sl], a_t, x_t[:, sl],
                                             start=(i == 0), stop=(i == n_mm - 1))
                            i += 1
                uf = ld.tile([3 * R, T], F32, name="uf")
                nc.any.tensor_copy(uf, ups)
                for pi, p in enumerate("qkv"):
                    upf = ld.tile([R, T], F32, name=f"u{p}f")
                    nc.sync.dma_start(out=upf, in_=uf[pi * R:(pi + 1) * R, :])
                    uh = ld.tile([R, T], BF16, name=f"u{p}h")
                    ul = ld.tile([R, T], BF16, name=f"u{p}l")
                    nc.vector.tensor_copy(uh, upf)
                    nc.vector.tensor_sub(ul, upf, uh)
                    u_bf[p + "h"], u_bf[p + "l"] = uh, ul

            # --- projections, transposed layout [OL, T] ---
            for p in "qkv":
                wh_t, wl_t = [], []
                for ci in range(CI):
                    t_ = wpool.tile([128, OL], BF16, tag=f"wh{ci}", name=f"wh{ci}")
                    nc.sync.dma_start(out=t_, in_=w_d[p + "h"][ts(ci, 128), :])
                    wh_t.append(t_)
                    if p != "v":
                        t_ = wpool.tile([128, OL], BF16, tag=f"wl{ci}", name=f"wl{ci}")
                        nc.sync.dma_start(out=t_, in_=w_d[p + "l"][ts(ci, 128), :])
                        wl_t.append(t_)
                vth_t = None
                if p == "v":
                    vth_t = [wpool.tile([128, T], BF16, tag=f"vth{c}", name=f"vth{c}",
                                        bufs=1) for c in range(2)]
                for oc in range(2):
                    osl = ts(oc, 128)
                    h0, h1 = 2 * oc, 2 * oc + 1
                    for ns in range(NS):
                        sl = ts(ns, 512)
                        ps = pps.tile([128, 512], F32, tag="proj", name="proj")
                        if p == "v":
                            base = [(wh_t[ci], xth_t[ci]) for ci in range(CI)]
                        else:
                            base = []
                            for ci in range(CI):
                                base += [(wh_t[ci], xth_t[ci]), (wh_t[ci], xtl_t[ci]),
                                         (wl_t[ci], xth_t[ci])]
                        seq = [(a[:, osl], b_[:, sl]) for a, b_ in base]
                        if lora:
                            seq += [(b_t[p + "h"][:, osl], u_bf[p + "h"][:, sl]),
                                    (b_t[p + "h"][:, osl], u_bf[p + "l"][:, sl]),
                                    (b_t[p + "l"][:, osl], u_bf[p + "h"][:, sl])]
                        for i, (a, b_) in enumerate(seq):
                            nc.tensor.matmul(ps, a, b_, start=(i == 0),
                                             stop=(i == len(seq) - 1))
                        if p == "v":
                            nc.any.tensor_copy(vth_t[oc][:, sl], ps)
                        elif p == "q":
                            for h, rows in ((h0, ps[0:64, :]), (h1, ps[64:128, :])):
                                nc.any.tensor_copy(qhh[h][0:64, sl], rows)
                                nc.any.tensor_copy(qhh[h][64:128, sl], rows)
                                nc.vector.tensor_sub(qlo[h][:, sl], rows,
                                                     qhh[h][0:64, sl])
                        else:
                            for h, rows in ((h0, ps[0:64, :]), (h1, ps[64:128, :])):
                                nc.any.tensor_copy(khl[h][0:64, sl], rows)
                                nc.vector.tensor_sub(khl[h][64:128, sl], rows,
                                                     khl[h][0:64, sl])
                if p == "v":
                    # v16 tiles: per head 64 v-cols + a ones column (Z trick)
                    for tci in range(TC):
                        nc.vector.memset(v16[tci], 1.0)
                    for oc in range(2):
                        for tci in range(TC):
                            tp = vtrp.tile([128, 128], BF16, tag="vtr", name="vtr")
                            nc.tensor.transpose(tp, vth_t[oc][:, ts(tci, 128)], ident)
                            h0, h1 = 2 * oc, 2 * oc + 1
                            nc.any.tensor_copy(v16[tci][:, h0 * VW:h0 * VW + 64],
                                               tp[:, 0:64])
                            nc.any.tensor_copy(v16[tci][:, h1 * VW:h1 * VW + 64],
                                               tp[:, 64:128])

        # ---------------- phase 3: attention ----------------
        with ExitStack() as ph3:
            att = ph3.enter_context(tc.tile_pool(name="att", bufs=2))
            ptp = ph3.enter_context(tc.tile_pool(name="ptp", bufs=2))
            scr = ph3.enter_context(tc.tile_pool(name="scr", bufs=4))
            drp = ph3.enter_context(tc.tile_pool(name="drp", bufs=2, space="DRAM"))
            msp = ph3.enter_context(tc.tile_pool(name="msp", bufs=1, space="PSUM"))
            sps = ph3.enter_context(tc.tile_pool(name="sps", bufs=3, space="PSUM"))
            cps = ph3.enter_context(tc.tile_pool(name="cps", bufs=1, space="PSUM"))

            for h in range(NH):
                ch = h // 2
                pr = (h % 2) * 64
                # --- m-hat pass for the whole head: [q,k] single-bf16 scores ---
                mh_bcs = []
                for qb in range(NS):
                    rm4 = att.tile([128, 4], F32, name="rm4")
                    for ql_i in range(4):
                        qt = qb * 4 + ql_i
                        ms = msp.tile([128, T], F32, tag="ms", name="ms")
                        for ns in range(NS):
                            nc.tensor.matmul(ms[:, ts(ns, 512)],
                                             qhh[h][0:64, ts(qt, 128)],
                                             khl[h][0:64, ts(ns, 512)],
                                             start=True, stop=True)
                        nc.vector.reduce_max(out=rm4[:, ql_i:ql_i + 1], in_=ms,
                                             axis=AX.X)
                    rm4s = att.tile([128, 4], F32, name="rm4s")
                    nc.vector.tensor_scalar_mul(rm4s, rm4, 0.125)
                    dr = drp.tile([128, 4], F32, name="mh_dr")
                    nc.sync.dma_start(out=dr, in_=rm4s)
                    # transpose via DRAM bounce: mh_row[0, ql_i*128+q] = dr[q, ql_i]
                    mh_row = att.tile([1, 512], F32, name="mh_row", bufs=4)
                    src = bass.AP(tensor=dr.tensor, offset=dr.offset,
                                  ap=[[1, 4], [4, 128]])
                    nc.sync.dma_start(out=mh_row, in_=src)
                    mh_bc = att.tile([128, 512], F32, name="mh_bc", bufs=4)
                    nc.gpsimd.partition_broadcast(mh_bc, mh_row, channels=128)
                    mh_bcs.append(mh_bc)

                for qb in range(NS):
                    qsl = ts(qb, 512)
                    mh_bc = mh_bcs[qb]
                    # --- sT pass: K-stacked split-2 scores -> stt -> exp -> pT ---
                    pT = [ptp.tile([128, 512], BF16, tag=f"pt{i}", name=f"pt{i}")
                          for i in range(TC)]
                    for kt in range(TC):
                        st = sps.tile([128, 512], F32, tag="st", name="st")
                        # kh·qh + kl·qh in one K=128 matmul (qh duplicated)
                        nc.tensor.matmul(st, khl[h][:, ts(kt, 128)], qhh[h][:, qsl],
                                         start=True, stop=False)
                        # kh·ql, K=64
                        nc.tensor.matmul(st, khl[h][0:64, ts(kt, 128)],
                                         qlo[h][:, qsl], start=False, stop=True)
                        sm = scr.tile([128, 512], BF16, tag="sm", name="sm")
                        nc.vector.scalar_tensor_tensor(out=sm, in0=st, scalar=0.125,
                                                       in1=mh_bc, op0=MULT, op1=SUB)
                        nc.scalar.activation(out=pT[kt], in_=sm, func=Exp)
                    # --- PV with ones column ---
                    cxa = cps.tile([VW, 512], F32, tag="cxa", name="cxa")
                    for kt in range(TC):
                        nc.tensor.matmul(cxa, v16[kt][:, h * VW:(h + 1) * VW], pT[kt],
                                         start=(kt == 0), stop=(kt == TC - 1))
                    # --- normalize by Z (row 64) during evacuation ---
                    zrow = att.tile([1, 512], F32, name="zrow")
                    nc.vector.tensor_copy(zrow, cxa[64:65, :])
                    z_bc = att.tile([64, 512], F32, name="z_bc")
                    nc.gpsimd.partition_broadcast(z_bc, zrow, channels=64)
                    rcp_bc = att.tile([64, 512], F32, name="rcp_bc")
                    nc.vector.reciprocal(rcp_bc, z_bc)
                    nc.vector.tensor_mul(ctxT_t[ch][pr:pr + 64, qsl], cxa[0:64, :],
                                         rcp_bc)

        # ---------------- phase 4: output projection ----------------
        with ExitStack() as ph4:
            ost_p = ph4.enter_context(tc.tile_pool(name="ost", bufs=3))
            ops = ph4.enter_context(tc.tile_pool(name="ops", bufs=2, space="PSUM"))
            woT_t = []
            for cc in range(2):
                t_ = ost_p.tile([128, E], BF16, tag=f"woT{cc}", name=f"woT{cc}")
                nc.sync.dma_start(out=t_, in_=woT_d[ts(cc, 128), :])
                woT_t.append(t_)
            for tci in range(TC):
                tsl = ts(tci, 128)
                ost = ost_p.tile([128, E], F32, tag="ost", name="ost")
                for no in range(2):
                    op_t = ops.tile([128, 512], F32, tag=f"op{no}", name=f"op{no}")
                    for cc in range(2):
                        nc.tensor.matmul(op_t, ctxT_t[cc][:, tsl],
                                         woT_t[cc][:, ts(no, 512)],
                                         start=(cc == 0), stop=(cc == 1))
                    nc.any.tensor_copy(ost[:, ts(no, 512)], op_t)
                nc.sync.dma_start(out=outp_d[tsl, :], in_=ost)

    nc.compile()
    _CACHE[key] = nc
    return nc


def _split(a):
    h = a.astype(bf16)
    l = (a - h.astype(np.float32)).astype(bf16)
    return h, l


def _shard(inputs, lora):
    x = np.asarray(inputs["x"], np.float32)
    Wo = np.asarray(inputs["Wo"], np.float32)
    ident = np.eye(128, dtype=np.float32).astype(bf16)
    if lora:
        A_all = np.concatenate([np.asarray(inputs["Aq"], np.float32),
                                np.asarray(inputs["Ak"], np.float32),
                                np.asarray(inputs["Av"], np.float32)], axis=1)
        ah, al = _split(A_all)
    in_maps = []
    for core in range(8):
        b, hp = core // 4, core % 4
        o0 = hp * OL
        xT = np.ascontiguousarray(x[b].T)
        xh, xl = _split(xT)
        m = {"xth": xh, "xtl": xl, "ident": ident}
        for p in "qkv":
            W = np.asarray(inputs["W" + p], np.float32)
            Ws = np.ascontiguousarray(W[o0:o0 + OL, :].T)
            m["w%sh" % p], m["w%sl" % p] = _split(Ws)
            if lora:
                B = np.asarray(inputs["B" + p], np.float32)[:, o0:o0 + OL] * 2.0
                m["b%sh" % p], m["b%sl" % p] = _split(B)
        m["woT"] = np.ascontiguousarray(Wo[:, o0:o0 + OL].T).astype(bf16)
        if lora:
            m["ah"], m["al"] = ah, al
        in_maps.append(m)
    return in_maps


def _run(inputs, trace=False, **kw):
    lora = not all(
        np.count_nonzero(np.asarray(inputs["B" + p])) == 0 for p in "qkv")
    nc = _build(lora)
    in_maps = _shard(inputs, lora)
    res = run_bass_kernel_spmd(nc, in_maps, core_ids=list(range(8)), trace=trace, **kw)
    bo = np.asarray(inputs["bo"], np.float32)
    parts = [res.results[c]["outp"].astype(np.float64) for c in range(8)]
    out = np.stack([sum(parts[0:4]), sum(parts[4:8])]) + bo.astype(np.float64)
    return out.astype(np.float32), res


def kernel(**inputs):
    out, _ = _run(inputs)
    return out


# revision 11
# speedup vs baseline: 1.5145x; 1.0467x over previous
"""LoRA self-attention TRN2 kernel (8 NeuronCores, SPMD) — v4.

Sharding: core c = (b, hp) with b = c // 4 (batch), hp = c % 4 (head group of
4 heads = 256 channels). Each core computes q/k/v projections (+LoRA) for its
256 output channels from the full x[b], runs attention for its 4 heads, and a
partial output projection over its 256 context channels. Host sums the 4
partials per batch element and adds bo.

Numerics: q/k projections and the [k,q]-oriented QK^T scores use bf16 hi/lo
splits (s = kh·qh + kl·qh + kh·ql, fp32-grade); the softmax shift m-hat comes
from a single-bf16 [q,k] score pass (error ≪ the exp-safety slack; the shift
cancels exactly in softmax). P·V and the output projection run in bf16.

PE-row reduction tricks (the chip power-caps sustained 8-core PE activity, so
wall time tracks streamed matmul rows):
  - K-stacked split: per head, K tiles hold [k_hi; k_lo] on 128 partitions and
    q_hi is duplicated on both halves, so kh·qh + kl·qh is ONE K=128 matmul;
    only kh·ql needs a second K=64 matmul.
  - ones-column on V makes PV row 64 the softmax normalizer Z (no reduce).
  - m-hat subtraction is fused into the PSUM evacuation on DVE
    (scalar_tensor_tensor), exp runs on ACT from SBUF bf16.
  - when every LoRA B factor is zero (standard LoRA init), a specialized
    no-LoRA program is compiled and used; the general path handles B != 0.
"""
import sys

sys.path.insert(0, "/opt/trn_rl_repo")

from contextlib import ExitStack

import numpy as np
import ml_dtypes

import concourse.bass as bass
import concourse.tile as tile
from concourse import bacc, mybir
from concourse.bass import ts
from concourse.bass_utils import run_bass_kernel_spmd

F32 = mybir.dt.float32
BF16 = mybir.dt.bfloat16
bf16 = ml_dtypes.bfloat16
AX = mybir.AxisListType
Exp = mybir.ActivationFunctionType.Exp
MULT = mybir.AluOpType.mult
SUB = mybir.AluOpType.subtract

T = 2048          # sequence length
E = 1024          # embed
OL = 256          # local output channels (4 heads)
D = 64            # head dim
NH = 4            # local heads
R = 8             # lora rank
CI = 8            # contraction chunks of 128 over E
NS = 4            # 512-wide slices over T
TC = 16           # 128-wide tiles over T
VW = 65           # v-aug width per head (64 + ones column)

_CACHE = {}


def _build(lora=True):
    key = ("nc", lora)
    if key in _CACHE:
        return _CACHE[key]

    nc = bacc.Bacc("TRN2", target_bir_lowering=False, debug=False)

    # ---- DRAM I/O ----
    xth_d = nc.dram_tensor("xth", [E, T], BF16, kind="ExternalInput")
    xtl_d = nc.dram_tensor("xtl", [E, T], BF16, kind="ExternalInput")
    w_d = {}
    for p in "qkv":
        for s in "hl":
            w_d[p + s] = nc.dram_tensor(f"w{p}{s}", [E, OL], BF16, kind="ExternalInput")
    woT_d = nc.dram_tensor("woT", [OL, E], BF16, kind="ExternalInput")
    if lora:
        ah_d = nc.dram_tensor("ah", [E, 3 * R], BF16, kind="ExternalInput")
        al_d = nc.dram_tensor("al", [E, 3 * R], BF16, kind="ExternalInput")
        b_d = {}
        for p in "qkv":
            for s in "hl":
                b_d[p + s] = nc.dram_tensor(f"b{p}{s}", [R, OL], BF16,
                                            kind="ExternalInput")
    ident_d = nc.dram_tensor("ident", [128, 128], BF16, kind="ExternalInput")
    outp_d = nc.dram_tensor("outp", [T, E], F32, kind="ExternalOutput")

    with tile.TileContext(nc) as tc, ExitStack() as ctx:
        # ---------------- persistent tiles ----------------
        # Per-head score operand layouts:
        #   khl[h] [128,T]: rows 0:64 = kT_hi(h), rows 64:128 = kT_lo(h)
        #   qhh[h] [128,T]: rows 0:64 = qT_hi(h), rows 64:128 = qT_hi(h) (dup)
        #   qlo[h] [64,T]:  qT_lo(h)
        pers = ctx.enter_context(tc.tile_pool(name="pers", bufs=1))
        khl = [pers.tile([128, T], BF16, name=f"khl{h}") for h in range(NH)]
        kha = [pers.tile([65, T], BF16, name=f"kha{h}") for h in range(NH)]
        qhh = [pers.tile([128, T], BF16, name=f"qhh{h}") for h in range(NH)]
        qla = [pers.tile([65, T], BF16, name=f"qla{h}") for h in range(NH)]
        v16 = [pers.tile([128, NH * VW], BF16, name=f"v16_{i}") for i in range(TC)]
        ctxT_t = [pers.tile([128, T], BF16, name=f"ctxT{c}") for c in range(2)]

        # ---------------- phase 1: projections ----------------
        with ExitStack() as ph1:
            ld = ph1.enter_context(tc.tile_pool(name="ld", bufs=1))
            wpool = ph1.enter_context(tc.tile_pool(name="wpool", bufs=2))
            pps = ph1.enter_context(tc.tile_pool(name="pps", bufs=2, space="PSUM"))
            upsp = ph1.enter_context(tc.tile_pool(name="upsp", bufs=1, space="PSUM"))
            vtrp = ph1.enter_context(tc.tile_pool(name="vtrp", bufs=2, space="PSUM"))

            ident = ld.tile([128, 128], BF16, name="ident")
            nc.sync.dma_start(out=ident, in_=ident_d[:, :])
            for h in range(NH):
                nc.vector.memset(kha[h][64:65, :], 1.0)

            xth_t, xtl_t, ah_t, al_t = [], [], [], []
            for ci in range(CI):
                t_ = ld.tile([128, T], BF16, name=f"xth{ci}")
                nc.sync.dma_start(out=t_, in_=xth_d[ts(ci, 128), :])
                xth_t.append(t_)
                t_ = ld.tile([128, T], BF16, name=f"xtl{ci}")
                nc.sync.dma_start(out=t_, in_=xtl_d[ts(ci, 128), :])
                xtl_t.append(t_)
                if lora:
                    t_ = ld.tile([128, 3 * R], BF16, name=f"ah{ci}")
                    nc.sync.dma_start(out=t_, in_=ah_d[ts(ci, 128), :])
                    ah_t.append(t_)
                    t_ = ld.tile([128, 3 * R], BF16, name=f"al{ci}")
                    nc.sync.dma_start(out=t_, in_=al_d[ts(ci, 128), :])
                    al_t.append(t_)
            u_bf = {}
            b_t = {}
            if lora:
                for key2, d in b_d.items():
                    t_ = ld.tile([R, OL], BF16, name=f"b{key2}")
                    nc.sync.dma_start(out=t_, in_=d[:, :])
                    b_t[key2] = t_

                # u_all = x @ A_all (split3), shared M=24 pass
                ups = upsp.tile([3 * R, T], F32, name="ups")
                for ns in range(NS):
                    sl = ts(ns, 512)
                    n_mm = 3 * CI
                    i = 0
                    for ci in range(CI):
                        for a_t, x_t in ((ah_t[ci], xth_t[ci]), (ah_t[ci], xtl_t[ci]),
                                         (al_t[ci], xth_t[ci])):
                            nc.tensor.matmul(ups[:, sl], a_t, x_t[:, sl],
                                             start=(i == 0), stop=(i == n_mm - 1))
                            i += 1
                uf = ld.tile([3 * R, T], F32, name="uf")
                nc.any.tensor_copy(uf, ups)
                for pi, p in enumerate("qkv"):
                    upf = ld.tile([R, T], F32, tag="upf", name=f"u{p}f")
                    nc.sync.dma_start(out=upf, in_=uf[pi * R:(pi + 1) * R, :])
                    uh = ld.tile([R, T], BF16, name=f"u{p}h")
                    ul = ld.tile([R, T], BF16, name=f"u{p}l")
                    nc.vector.tensor_copy(uh, upf)
                    nc.vector.tensor_sub(ul, upf, uh)
                    u_bf[p + "h"], u_bf[p + "l"] = uh, ul

            # --- projections, transposed layout [OL, T] ---
            for p in "qkv":
                wh_t, wl_t = [], []
                for ci in range(CI):
                    t_ = wpool.tile([128, OL], BF16, tag=f"wh{ci}", name=f"wh{ci}")
                    nc.sync.dma_start(out=t_, in_=w_d[p + "h"][ts(ci, 128), :])
                    wh_t.append(t_)
                    if p != "v":
                        t_ = wpool.tile([128, OL], BF16, tag=f"wl{ci}", name=f"wl{ci}")
                        nc.sync.dma_start(out=t_, in_=w_d[p + "l"][ts(ci, 128), :])
                        wl_t.append(t_)
                vth_t = None
                if p == "v":
                    vth_t = [wpool.tile([128, T], BF16, tag=f"vth{c}", name=f"vth{c}",
                                        bufs=1) for c in range(2)]
                for oc in range(2):
                    osl = ts(oc, 128)
                    h0, h1 = 2 * oc, 2 * oc + 1
                    for ns in range(NS):
                        sl = ts(ns, 512)
                        ps = pps.tile([128, 512], F32, tag="proj", name="proj")
                        if p == "v":
                            base = [(wh_t[ci], xth_t[ci]) for ci in range(CI)]
                        else:
                            base = []
                            for ci in range(CI):
                                base += [(wh_t[ci], xth_t[ci]), (wh_t[ci], xtl_t[ci]),
                                         (wl_t[ci], xth_t[ci])]
                        seq = [(a[:, osl], b_[:, sl]) for a, b_ in base]
                        if lora:
                            seq += [(b_t[p + "h"][:, osl], u_bf[p + "h"][:, sl]),
                                    (b_t[p + "h"][:, osl], u_bf[p + "l"][:, sl]),
                                    (b_t[p + "l"][:, osl], u_bf[p + "h"][:, sl])]
                        for i, (a, b_) in enumerate(seq):
                            nc.tensor.matmul(ps, a, b_, start=(i == 0),
                                             stop=(i == len(seq) - 1))
                        if p == "v":
                            nc.any.tensor_copy(vth_t[oc][:, sl], ps)
                        elif p == "q":
                            for h, rows in ((h0, ps[0:64, :]), (h1, ps[64:128, :])):
                                nc.any.tensor_copy(qhh[h][0:64, sl], rows)
                                nc.any.tensor_copy(qhh[h][64:128, sl], rows)
                                nc.vector.tensor_sub(qla[h][0:64, sl], rows,
                                                     qhh[h][0:64, sl])
                        else:
                            for h, rows in ((h0, ps[0:64, :]), (h1, ps[64:128, :])):
                                nc.any.tensor_copy(khl[h][0:64, sl], rows)
                                nc.any.tensor_copy(kha[h][0:64, sl], rows)
                                nc.vector.tensor_sub(khl[h][64:128, sl], rows,
                                                     khl[h][0:64, sl])
                if p == "v":
                    # v16 tiles: per head 64 v-cols + a ones column (Z trick)
                    for tci in range(TC):
                        nc.vector.memset(v16[tci], 1.0)
                    for oc in range(2):
                        for tci in range(TC):
                            tp = vtrp.tile([128, 128], BF16, tag="vtr", name="vtr")
                            nc.tensor.transpose(tp, vth_t[oc][:, ts(tci, 128)], ident)
                            h0, h1 = 2 * oc, 2 * oc + 1
                            nc.any.tensor_copy(v16[tci][:, h0 * VW:h0 * VW + 64],
                                               tp[:, 0:64])
                            nc.any.tensor_copy(v16[tci][:, h1 * VW:h1 * VW + 64],
                                               tp[:, 64:128])

        # ---------------- phase 3: attention ----------------
        with ExitStack() as ph3:
            att = ph3.enter_context(tc.tile_pool(name="att", bufs=2))
            ptp = ph3.enter_context(tc.tile_pool(name="ptp", bufs=2))
            drp = ph3.enter_context(tc.tile_pool(name="drp", bufs=2, space="DRAM"))
            msp = ph3.enter_context(tc.tile_pool(name="msp", bufs=1, space="PSUM"))
            sps = ph3.enter_context(tc.tile_pool(name="sps", bufs=3, space="PSUM"))
            cps = ph3.enter_context(tc.tile_pool(name="cps", bufs=1, space="PSUM"))

            for h in range(NH):
                ch = h // 2
                pr = (h % 2) * 64
                # --- m-hat pass for the whole head: [q,k] single-bf16 scores;
                # -m-hat lands in qla[h] row 64 via a DRAM transpose bounce, so
                # the K=65 score matmul subtracts it inside PSUM for free ---
                rm16 = att.tile([128, 16], F32, name="rm16")
                for qt in range(TC):
                    ms = msp.tile([128, T], F32, tag="ms", name="ms")
                    for ns in range(NS):
                        nc.tensor.matmul(ms[:, ts(ns, 512)],
                                         qhh[h][0:64, ts(qt, 128)],
                                         khl[h][0:64, ts(ns, 512)],
                                         start=True, stop=True)
                    nc.vector.reduce_max(out=rm16[:, qt:qt + 1], in_=ms, axis=AX.X)
                rm16s = att.tile([128, 16], BF16, name="rm16s")
                nc.vector.tensor_scalar_mul(rm16s, rm16, -1.0)
                dr = drp.tile([128, 16], BF16, name="mh_dr")
                nc.sync.dma_start(out=dr, in_=rm16s)
                # transpose via DRAM bounce: qla[h][64, qt*128+q] = dr[q, qt]
                src = bass.AP(tensor=dr.tensor, offset=dr.offset,
                              ap=[[1, 16], [16, 128]])
                nc.sync.dma_start(out=qla[h][64:65, :], in_=src)

                for qb in range(NS):
                    qsl = ts(qb, 512)
                    # --- sT pass: K-stacked scores with fused -m-hat -> exp ---
                    pT = [ptp.tile([128, 512], BF16, tag=f"pt{i}", name=f"pt{i}")
                          for i in range(TC)]
                    for kt in range(TC):
                        st = sps.tile([128, 512], F32, tag="st", name="st")
                        # kh·qh + kl·qh in one K=128 matmul (qh duplicated)
                        nc.tensor.matmul(st, khl[h][:, ts(kt, 128)], qhh[h][:, qsl],
                                         start=True, stop=False)
                        # kh·ql + ones·(-m-hat), K=65
                        nc.tensor.matmul(st, kha[h][:, ts(kt, 128)],
                                         qla[h][:, qsl], start=False, stop=True)
                        nc.scalar.activation(out=pT[kt], in_=st, func=Exp, scale=0.125)
                    # --- PV with ones column ---
                    cxa = cps.tile([VW, 512], F32, tag="cxa", name="cxa")
                    for kt in range(TC):
                        nc.tensor.matmul(cxa, v16[kt][:, h * VW:(h + 1) * VW], pT[kt],
                                         start=(kt == 0), stop=(kt == TC - 1))
                    # --- normalize by Z (row 64) during evacuation ---
                    zrow = att.tile([1, 512], F32, name="zrow")
                    nc.vector.tensor_copy(zrow, cxa[64:65, :])
                    z_bc = att.tile([64, 512], F32, name="z_bc")
                    nc.gpsimd.partition_broadcast(z_bc, zrow, channels=64)
                    rcp_bc = att.tile([64, 512], F32, name="rcp_bc")
                    nc.vector.reciprocal_approx_fast(out=rcp_bc, in_=z_bc)
                    nc.vector.tensor_mul(ctxT_t[ch][pr:pr + 64, qsl], cxa[0:64, :],
                                         rcp_bc)

        # ---------------- phase 4: output projection ----------------
        with ExitStack() as ph4:
            ost_p = ph4.enter_context(tc.tile_pool(name="ost", bufs=3))
            ops = ph4.enter_context(tc.tile_pool(name="ops", bufs=2, space="PSUM"))
            woT_t = []
            for cc in range(2):
                t_ = ost_p.tile([128, E], BF16, tag=f"woT{cc}", name=f"woT{cc}")
                nc.sync.dma_start(out=t_, in_=woT_d[ts(cc, 128), :])
                woT_t.append(t_)
            for tci in range(TC):
                tsl = ts(tci, 128)
                ost = ost_p.tile([128, E], F32, tag="ost", name="ost")
                for no in range(2):
                    op_t = ops.tile([128, 512], F32, tag=f"op{no}", name=f"op{no}")
                    for cc in range(2):
                        nc.tensor.matmul(op_t, ctxT_t[cc][:, tsl],
                                         woT_t[cc][:, ts(no, 512)],
                                         start=(cc == 0), stop=(cc == 1))
                    nc.any.tensor_copy(ost[:, ts(no, 512)], op_t)
                nc.sync.dma_start(out=outp_d[tsl, :], in_=ost)

    nc.compile()
    _CACHE[key] = nc
    return nc


def _split(a):
    h = a.astype(bf16)
    l = (a - h.astype(np.float32)).astype(bf16)
    return h, l


def _shard(inputs, lora):
    x = np.asarray(inputs["x"], np.float32)
    Wo = np.asarray(inputs["Wo"], np.float32)
    ident = np.eye(128, dtype=np.float32).astype(bf16)
    if lora:
        A_all = np.concatenate([np.asarray(inputs["Aq"], np.float32),
                                np.asarray(inputs["Ak"], np.float32),
                                np.asarray(inputs["Av"], np.float32)], axis=1)
        ah, al = _split(A_all)
    in_maps = []
    for core in range(8):
        b, hp = core // 4, core % 4
        o0 = hp * OL
        xT = np.ascontiguousarray(x[b].T)
        xh, xl = _split(xT)
        m = {"xth": xh, "xtl": xl, "ident": ident}
        for p in "qkv":
            W = np.asarray(inputs["W" + p], np.float32)
            Ws = np.ascontiguousarray(W[o0:o0 + OL, :].T)
            m["w%sh" % p], m["w%sl" % p] = _split(Ws)
            if lora:
                B = np.asarray(inputs["B" + p], np.float32)[:, o0:o0 + OL] * 2.0
                m["b%sh" % p], m["b%sl" % p] = _split(B)
        m["woT"] = np.ascontiguousarray(Wo[:, o0:o0 + OL].T).astype(bf16)
        if lora:
            m["ah"], m["al"] = ah, al
        in_maps.append(m)
    return in_maps


def _run(inputs, trace=False, **kw):
    lora = not all(
        np.count_nonzero(np.asarray(inputs["B" + p])) == 0 for p in "qkv")
    nc = _build(lora)
    in_maps = _shard(inputs, lora)
    res = run_bass_kernel_spmd(nc, in_maps, core_ids=list(range(8)), trace=trace, **kw)
    bo = np.asarray(inputs["bo"], np.float32)
    parts = [res.results[c]["outp"].astype(np.float64) for c in range(8)]
    out = np.stack([sum(parts[0:4]), sum(parts[4:8])]) + bo.astype(np.float64)
    return out.astype(np.float32), res


def kernel(**inputs):
    out, _ = _run(inputs)
    return out


# revision 12
# speedup vs baseline: 1.8278x; 1.2069x over previous
"""LoRA self-attention TRN2 kernel (8 NeuronCores, SPMD) — v4.

Sharding: core c = (b, hp) with b = c // 4 (batch), hp = c % 4 (head group of
4 heads = 256 channels). Each core computes q/k/v projections (+LoRA) for its
256 output channels from the full x[b], runs attention for its 4 heads, and a
partial output projection over its 256 context channels. Host sums the 4
partials per batch element and adds bo.

Numerics: q/k projections and the [k,q]-oriented QK^T scores use bf16 hi/lo
splits (s = kh·qh + kl·qh + kh·ql, fp32-grade); the softmax shift m-hat comes
from a single-bf16 [q,k] score pass (error ≪ the exp-safety slack; the shift
cancels exactly in softmax). P·V and the output projection run in bf16.

PE-row reduction tricks (the chip power-caps sustained 8-core PE activity, so
wall time tracks streamed matmul rows):
  - K-stacked split: per head, K tiles hold [k_hi; k_lo] on 128 partitions and
    q_hi is duplicated on both halves, so kh·qh + kl·qh is ONE K=128 matmul;
    only kh·ql needs a second K=64 matmul.
  - ones-column on V makes PV row 64 the softmax normalizer Z (no reduce).
  - m-hat subtraction is fused into the PSUM evacuation on DVE
    (scalar_tensor_tensor), exp runs on ACT from SBUF bf16.
  - when every LoRA B factor is zero (standard LoRA init), a specialized
    no-LoRA program is compiled and used; the general path handles B != 0.
"""
import sys

sys.path.insert(0, "/opt/trn_rl_repo")

from contextlib import ExitStack

import numpy as np
import ml_dtypes

import concourse.bass as bass
import concourse.tile as tile
from concourse import bacc, mybir
from concourse.bass import ts
from concourse.bass_utils import run_bass_kernel_spmd

F32 = mybir.dt.float32
BF16 = mybir.dt.bfloat16
bf16 = ml_dtypes.bfloat16
AX = mybir.AxisListType
Exp = mybir.ActivationFunctionType.Exp
MULT = mybir.AluOpType.mult
SUB = mybir.AluOpType.subtract

T = 2048          # sequence length
E = 1024          # embed
OL = 256          # local output channels (4 heads)
D = 64            # head dim
NH = 4            # local heads
R = 8             # lora rank
CI = 8            # contraction chunks of 128 over E
NS = 4            # 512-wide slices over T
TC = 16           # 128-wide tiles over T
VW = 65           # v-aug width per head (64 + ones column)

_CACHE = {}


def _build(lora=True):
    key = ("nc", lora)
    if key in _CACHE:
        return _CACHE[key]

    nc = bacc.Bacc("TRN2", target_bir_lowering=False, debug=False)

    # ---- DRAM I/O ----
    xth_d = nc.dram_tensor("xth", [E, T], BF16, kind="ExternalInput")
    xtl_d = nc.dram_tensor("xtl", [E, T], BF16, kind="ExternalInput")
    w_d = {}
    for p in "qkv":
        for s in "hl":
            w_d[p + s] = nc.dram_tensor(f"w{p}{s}", [E, OL], BF16, kind="ExternalInput")
    woT_d = nc.dram_tensor("woT", [OL, E], BF16, kind="ExternalInput")
    if lora:
        ah_d = nc.dram_tensor("ah", [E, 3 * R], BF16, kind="ExternalInput")
        al_d = nc.dram_tensor("al", [E, 3 * R], BF16, kind="ExternalInput")
        b_d = {}
        for p in "qkv":
            for s in "hl":
                b_d[p + s] = nc.dram_tensor(f"b{p}{s}", [R, OL], BF16,
                                            kind="ExternalInput")
    ident_d = nc.dram_tensor("ident", [128, 128], BF16, kind="ExternalInput")
    outp_d = nc.dram_tensor("outp", [T, E], F32, kind="ExternalOutput")

    with tile.TileContext(nc) as tc, ExitStack() as ctx:
        # ---------------- persistent tiles ----------------
        # Per-head score operand layouts:
        #   khl[h] [128,T]: rows 0:64 = kT_hi(h), rows 64:128 = kT_lo(h)
        #   qhh[h] [128,T]: rows 0:64 = qT_hi(h), rows 64:128 = qT_hi(h) (dup)
        #   qlo[h] [64,T]:  qT_lo(h)
        pers = ctx.enter_context(tc.tile_pool(name="pers", bufs=1))
        khl = [pers.tile([128, T], BF16, name=f"khl{h}") for h in range(NH)]
        kha = [pers.tile([65, T], BF16, name=f"kha{h}") for h in range(NH)]
        qhh = [pers.tile([128, T], BF16, name=f"qhh{h}") for h in range(NH)]
        qla = [pers.tile([65, T], BF16, name=f"qla{h}") for h in range(NH)]
        v16 = [pers.tile([128, NH * VW], BF16, name=f"v16_{i}") for i in range(TC)]
        ctxT_t = [pers.tile([128, T], BF16, name=f"ctxT{c}") for c in range(2)]

        # ---------------- phase 1: projections ----------------
        with ExitStack() as ph1:
            ld = ph1.enter_context(tc.tile_pool(name="ld", bufs=1))
            wpool = ph1.enter_context(tc.tile_pool(name="wpool", bufs=2))
            pps = ph1.enter_context(tc.tile_pool(name="pps", bufs=2, space="PSUM"))
            upsp = ph1.enter_context(tc.tile_pool(name="upsp", bufs=1, space="PSUM"))
            vtrp = ph1.enter_context(tc.tile_pool(name="vtrp", bufs=2, space="PSUM"))

            ident = ld.tile([128, 128], BF16, name="ident")
            nc.sync.dma_start(out=ident, in_=ident_d[:, :])
            for h in range(NH):
                nc.vector.memset(kha[h][64:65, :], 1.0)

            xth_t, xtl_t, ah_t, al_t = [], [], [], []
            for ci in range(CI):
                t_ = ld.tile([128, T], BF16, name=f"xth{ci}")
                nc.sync.dma_start(out=t_, in_=xth_d[ts(ci, 128), :])
                xth_t.append(t_)
                t_ = ld.tile([128, T], BF16, name=f"xtl{ci}")
                nc.sync.dma_start(out=t_, in_=xtl_d[ts(ci, 128), :])
                xtl_t.append(t_)
                if lora:
                    t_ = ld.tile([128, 3 * R], BF16, name=f"ah{ci}")
                    nc.sync.dma_start(out=t_, in_=ah_d[ts(ci, 128), :])
                    ah_t.append(t_)
                    t_ = ld.tile([128, 3 * R], BF16, name=f"al{ci}")
                    nc.sync.dma_start(out=t_, in_=al_d[ts(ci, 128), :])
                    al_t.append(t_)
            u_bf = {}
            b_t = {}
            if lora:
                for key2, d in b_d.items():
                    t_ = ld.tile([R, OL], BF16, name=f"b{key2}")
                    nc.sync.dma_start(out=t_, in_=d[:, :])
                    b_t[key2] = t_

                # u_all = x @ A_all (split3), shared M=24 pass
                ups = upsp.tile([3 * R, T], F32, name="ups")
                for ns in range(NS):
                    sl = ts(ns, 512)
                    n_mm = 3 * CI
                    i = 0
                    for ci in range(CI):
                        for a_t, x_t in ((ah_t[ci], xth_t[ci]), (ah_t[ci], xtl_t[ci]),
                                         (al_t[ci], xth_t[ci])):
                            nc.tensor.matmul(ups[:, sl], a_t, x_t[:, sl],
                                             start=(i == 0), stop=(i == n_mm - 1))
                            i += 1
                uf = ld.tile([3 * R, T], F32, name="uf")
                nc.any.tensor_copy(uf, ups)
                for pi, p in enumerate("qkv"):
                    upf = ld.tile([R, T], F32, tag="upf", name=f"u{p}f")
                    nc.sync.dma_start(out=upf, in_=uf[pi * R:(pi + 1) * R, :])
                    uh = ld.tile([R, T], BF16, name=f"u{p}h")
                    ul = ld.tile([R, T], BF16, name=f"u{p}l")
                    nc.vector.tensor_copy(uh, upf)
                    nc.vector.tensor_sub(ul, upf, uh)
                    u_bf[p + "h"], u_bf[p + "l"] = uh, ul

            # --- projections, transposed layout [OL, T] ---
            for p in "qkv":
                wh_t, wl_t = [], []
                for ci in range(CI):
                    t_ = wpool.tile([128, OL], BF16, tag=f"wh{ci}", name=f"wh{ci}")
                    nc.sync.dma_start(out=t_, in_=w_d[p + "h"][ts(ci, 128), :])
                    wh_t.append(t_)
                    if p != "v":
                        t_ = wpool.tile([128, OL], BF16, tag=f"wl{ci}", name=f"wl{ci}")
                        nc.sync.dma_start(out=t_, in_=w_d[p + "l"][ts(ci, 128), :])
                        wl_t.append(t_)
                vth_t = None
                if p == "v":
                    vth_t = [wpool.tile([128, T], BF16, tag=f"vth{c}", name=f"vth{c}",
                                        bufs=1) for c in range(2)]
                for oc in range(2):
                    osl = ts(oc, 128)
                    h0, h1 = 2 * oc, 2 * oc + 1
                    for ns in range(NS):
                        sl = ts(ns, 512)
                        ps = pps.tile([128, 512], F32, tag="proj", name="proj")
                        if p == "v":
                            base = [(wh_t[ci], xth_t[ci]) for ci in range(CI)]
                        else:
                            base = []
                            for ci in range(CI):
                                base += [(wh_t[ci], xth_t[ci]), (wh_t[ci], xtl_t[ci]),
                                         (wl_t[ci], xth_t[ci])]
                        seq = [(a[:, osl], b_[:, sl]) for a, b_ in base]
                        if lora:
                            seq += [(b_t[p + "h"][:, osl], u_bf[p + "h"][:, sl]),
                                    (b_t[p + "h"][:, osl], u_bf[p + "l"][:, sl]),
                                    (b_t[p + "l"][:, osl], u_bf[p + "h"][:, sl])]
                        for i, (a, b_) in enumerate(seq):
                            nc.tensor.matmul(ps, a, b_, start=(i == 0),
                                             stop=(i == len(seq) - 1))
                        if p == "v":
                            nc.any.tensor_copy(vth_t[oc][:, sl], ps)
                        elif p == "q":
                            for h, rows in ((h0, ps[0:64, :]), (h1, ps[64:128, :])):
                                nc.any.tensor_copy(qhh[h][0:64, sl], rows)
                                nc.any.tensor_copy(qhh[h][64:128, sl], rows)
                                nc.vector.tensor_sub(qla[h][0:64, sl], rows,
                                                     qhh[h][0:64, sl])
                        else:
                            for h, rows in ((h0, ps[0:64, :]), (h1, ps[64:128, :])):
                                nc.any.tensor_copy(khl[h][0:64, sl], rows)
                                nc.any.tensor_copy(kha[h][0:64, sl], rows)
                                nc.vector.tensor_sub(khl[h][64:128, sl], rows,
                                                     khl[h][0:64, sl])
                if p == "v":
                    # v16 tiles: per head 64 v-cols + a ones column (Z trick)
                    for tci in range(TC):
                        nc.vector.memset(v16[tci], 1.0)
                    for oc in range(2):
                        for tci in range(TC):
                            tp = vtrp.tile([128, 128], BF16, tag="vtr", name="vtr")
                            nc.tensor.transpose(tp, vth_t[oc][:, ts(tci, 128)], ident)
                            h0, h1 = 2 * oc, 2 * oc + 1
                            nc.any.tensor_copy(v16[tci][:, h0 * VW:h0 * VW + 64],
                                               tp[:, 0:64])
                            nc.any.tensor_copy(v16[tci][:, h1 * VW:h1 * VW + 64],
                                               tp[:, 64:128])

        # ---------------- phase 3: attention ----------------
        with ExitStack() as ph3:
            att = ph3.enter_context(tc.tile_pool(name="att", bufs=2))
            ptp = ph3.enter_context(tc.tile_pool(name="ptp", bufs=2))
            drp = ph3.enter_context(tc.tile_pool(name="drp", bufs=2, space="DRAM"))
            msp = ph3.enter_context(tc.tile_pool(name="msp", bufs=1, space="PSUM"))
            sps = ph3.enter_context(tc.tile_pool(name="sps", bufs=3, space="PSUM"))
            cps = ph3.enter_context(tc.tile_pool(name="cps", bufs=1, space="PSUM"))

            def mhat_pass(h):
                # m-hat for head h: [q,k] single-bf16 scores -> row max ->
                # -m-hat lands in qla[h] row 64 via a DRAM transpose bounce, so
                # the K=65 score matmul subtracts it inside PSUM for free
                rm16 = att.tile([128, 16], F32, name="rm16")
                for qt in range(TC):
                    ms = msp.tile([128, T], F32, tag="ms", name="ms")
                    for ns in range(NS):
                        nc.tensor.matmul(ms[:, ts(ns, 512)],
                                         qhh[h][0:64, ts(qt, 128)],
                                         khl[h][0:64, ts(ns, 512)],
                                         start=True, stop=True)
                    nc.vector.reduce_max(out=rm16[:, qt:qt + 1], in_=ms, axis=AX.X)
                rm16s = att.tile([128, 16], BF16, name="rm16s")
                nc.vector.tensor_scalar_mul(rm16s, rm16, -1.0)
                dr = drp.tile([128, 16], BF16, name="mh_dr")
                nc.sync.dma_start(out=dr, in_=rm16s)
                # transpose via DRAM bounce: qla[h][64, qt*128+q] = dr[q, qt]
                src = bass.AP(tensor=dr.tensor, offset=dr.offset,
                              ap=[[1, 16], [16, 128]])
                nc.sync.dma_start(out=qla[h][64:65, :], in_=src)

            mhat_pass(0)
            for h in range(NH):
                ch = h // 2
                pr = (h % 2) * 64
                if h + 1 < NH:
                    mhat_pass(h + 1)   # pipelined one head ahead

                for qb in range(NS):
                    qsl = ts(qb, 512)
                    # --- sT pass: K-stacked scores with fused -m-hat -> exp ---
                    pT = [ptp.tile([128, 512], BF16, tag=f"pt{i}", name=f"pt{i}")
                          for i in range(TC)]
                    for kt in range(TC):
                        st = sps.tile([128, 512], F32, tag="st", name="st")
                        # kh·qh + kl·qh in one K=128 matmul (qh duplicated)
                        nc.tensor.matmul(st, khl[h][:, ts(kt, 128)], qhh[h][:, qsl],
                                         start=True, stop=False)
                        # kh·ql + ones·(-m-hat), K=65
                        nc.tensor.matmul(st, kha[h][:, ts(kt, 128)],
                                         qla[h][:, qsl], start=False, stop=True)
                        nc.scalar.activation(out=pT[kt], in_=st, func=Exp, scale=0.125)
                    # --- PV with ones column ---
                    cxa = cps.tile([VW, 512], F32, tag="cxa", name="cxa")
                    for kt in range(TC):
                        nc.tensor.matmul(cxa, v16[kt][:, h * VW:(h + 1) * VW], pT[kt],
                                         start=(kt == 0), stop=(kt == TC - 1))
                    # --- normalize by Z (row 64) during evacuation ---
                    zrow = att.tile([1, 512], F32, name="zrow")
                    nc.vector.tensor_copy(zrow, cxa[64:65, :])
                    z_bc = att.tile([64, 512], F32, name="z_bc")
                    nc.gpsimd.partition_broadcast(z_bc, zrow, channels=64)
                    rcp_bc = att.tile([64, 512], F32, name="rcp_bc")
                    nc.vector.reciprocal_approx_fast(out=rcp_bc, in_=z_bc)
                    nc.vector.tensor_mul(ctxT_t[ch][pr:pr + 64, qsl], cxa[0:64, :],
                                         rcp_bc)

        # ---------------- phase 4: output projection ----------------
        with ExitStack() as ph4:
            ost_p = ph4.enter_context(tc.tile_pool(name="ost", bufs=3))
            ops = ph4.enter_context(tc.tile_pool(name="ops", bufs=2, space="PSUM"))
            woT_t = []
            for cc in range(2):
                t_ = ost_p.tile([128, E], BF16, tag=f"woT{cc}", name=f"woT{cc}")
                nc.sync.dma_start(out=t_, in_=woT_d[ts(cc, 128), :])
                woT_t.append(t_)
            for tci in range(TC):
                tsl = ts(tci, 128)
                ost = ost_p.tile([128, E], F32, tag="ost", name="ost")
                for no in range(2):
                    op_t = ops.tile([128, 512], F32, tag=f"op{no}", name=f"op{no}")
                    for cc in range(2):
                        nc.tensor.matmul(op_t, ctxT_t[cc][:, tsl],
                                         woT_t[cc][:, ts(no, 512)],
                                         start=(cc == 0), stop=(cc == 1))
                    nc.any.tensor_copy(ost[:, ts(no, 512)], op_t)
                nc.sync.dma_start(out=outp_d[tsl, :], in_=ost)

    nc.compile()
    _CACHE[key] = nc
    return nc


def _split(a):
    h = a.astype(bf16)
    l = (a - h.astype(np.float32)).astype(bf16)
    return h, l


def _shard(inputs, lora):
    x = np.asarray(inputs["x"], np.float32)
    Wo = np.asarray(inputs["Wo"], np.float32)
    ident = np.eye(128, dtype=np.float32).astype(bf16)
    if lora:
        A_all = np.concatenate([np.asarray(inputs["Aq"], np.float32),
                                np.asarray(inputs["Ak"], np.float32),
                                np.asarray(inputs["Av"], np.float32)], axis=1)
        ah, al = _split(A_all)
    in_maps = []
    for core in range(8):
        b, hp = core // 4, core % 4
        o0 = hp * OL
        xT = np.ascontiguousarray(x[b].T)
        xh, xl = _split(xT)
        m = {"xth": xh, "xtl": xl, "ident": ident}
        for p in "qkv":
            W = np.asarray(inputs["W" + p], np.float32)
            Ws = np.ascontiguousarray(W[o0:o0 + OL, :].T)
            m["w%sh" % p], m["w%sl" % p] = _split(Ws)
            if lora:
                B = np.asarray(inputs["B" + p], np.float32)[:, o0:o0 + OL] * 2.0
                m["b%sh" % p], m["b%sl" % p] = _split(B)
        m["woT"] = np.ascontiguousarray(Wo[:, o0:o0 + OL].T).astype(bf16)
        if lora:
            m["ah"], m["al"] = ah, al
        in_maps.append(m)
    return in_maps


def _run(inputs, trace=False, **kw):
    lora = not all(
        np.count_nonzero(np.asarray(inputs["B" + p])) == 0 for p in "qkv")
    nc = _build(lora)
    in_maps = _shard(inputs, lora)
    res = run_bass_kernel_spmd(nc, in_maps, core_ids=list(range(8)), trace=trace, **kw)
    bo = np.asarray(inputs["bo"], np.float32)
    parts = [res.results[c]["outp"].astype(np.float64) for c in range(8)]
    out = np.stack([sum(parts[0:4]), sum(parts[4:8])]) + bo.astype(np.float64)
    return out.astype(np.float32), res


def kernel(**inputs):
    out, _ = _run(inputs)
    return out


# revision 13
# speedup vs baseline: 1.9685x; 1.0770x over previous
"""LoRA self-attention TRN2 kernel (8 NeuronCores, SPMD) — v4.

Sharding: core c = (b, hp) with b = c // 4 (batch), hp = c % 4 (head group of
4 heads = 256 channels). Each core computes q/k/v projections (+LoRA) for its
256 output channels from the full x[b], runs attention for its 4 heads, and a
partial output projection over its 256 context channels. Host sums the 4
partials per batch element and adds bo.

Numerics: q/k projections and the [k,q]-oriented QK^T scores use bf16 hi/lo
splits (s = kh·qh + kl·qh + kh·ql, fp32-grade); the softmax shift m-hat comes
from a single-bf16 [q,k] score pass (error ≪ the exp-safety slack; the shift
cancels exactly in softmax). P·V and the output projection run in bf16.

PE-row reduction tricks (the chip power-caps sustained 8-core PE activity, so
wall time tracks streamed matmul rows):
  - K-stacked split: per head, K tiles hold [k_hi; k_lo] on 128 partitions and
    q_hi is duplicated on both halves, so kh·qh + kl·qh is ONE K=128 matmul;
    only kh·ql needs a second K=64 matmul.
  - ones-column on V makes PV row 64 the softmax normalizer Z (no reduce).
  - m-hat subtraction is fused into the PSUM evacuation on DVE
    (scalar_tensor_tensor), exp runs on ACT from SBUF bf16.
  - when every LoRA B factor is zero (standard LoRA init), a specialized
    no-LoRA program is compiled and used; the general path handles B != 0.
"""
import sys

sys.path.insert(0, "/opt/trn_rl_repo")

from contextlib import ExitStack

import numpy as np
import ml_dtypes

import concourse.bass as bass
import concourse.tile as tile
from concourse import bacc, mybir
from concourse.bass import ts
from concourse.bass_utils import run_bass_kernel_spmd

F32 = mybir.dt.float32
BF16 = mybir.dt.bfloat16
bf16 = ml_dtypes.bfloat16
AX = mybir.AxisListType
Exp = mybir.ActivationFunctionType.Exp
MULT = mybir.AluOpType.mult
SUB = mybir.AluOpType.subtract

T = 2048          # sequence length
E = 1024          # embed
OL = 256          # local output channels (4 heads)
D = 64            # head dim
NH = 4            # local heads
R = 8             # lora rank
CI = 8            # contraction chunks of 128 over E
NS = 4            # 512-wide slices over T
TC = 16           # 128-wide tiles over T
VW = 65           # v-aug width per head (64 + ones column)

_CACHE = {}


def _build(lora=True):
    key = ("nc", lora)
    if key in _CACHE:
        return _CACHE[key]

    nc = bacc.Bacc("TRN2", target_bir_lowering=False, debug=False)

    # ---- DRAM I/O ----
    xth_d = nc.dram_tensor("xth", [E, T], BF16, kind="ExternalInput")
    xtl_d = nc.dram_tensor("xtl", [E, T], BF16, kind="ExternalInput")
    w_d = {}
    for p in "qkv":
        for s in "hl":
            w_d[p + s] = nc.dram_tensor(f"w{p}{s}", [E, OL], BF16, kind="ExternalInput")
    woT_d = nc.dram_tensor("woT", [OL, E], BF16, kind="ExternalInput")
    if lora:
        ah_d = nc.dram_tensor("ah", [E, 3 * R], BF16, kind="ExternalInput")
        al_d = nc.dram_tensor("al", [E, 3 * R], BF16, kind="ExternalInput")
        b_d = {}
        for p in "qkv":
            for s in "hl":
                b_d[p + s] = nc.dram_tensor(f"b{p}{s}", [R, OL], BF16,
                                            kind="ExternalInput")
    ident_d = nc.dram_tensor("ident", [128, 128], BF16, kind="ExternalInput")
    outp_d = nc.dram_tensor("outp", [T, E], F32, kind="ExternalOutput")

    with tile.TileContext(nc) as tc, ExitStack() as ctx:
        # ---------------- persistent tiles ----------------
        # Per-head score operand layouts:
        #   khl[h] [128,T]: rows 0:64 = kT_hi(h), rows 64:128 = kT_lo(h)
        #   qhh[h] [128,T]: rows 0:64 = qT_hi(h), rows 64:128 = qT_hi(h) (dup)
        #   qlo[h] [64,T]:  qT_lo(h)
        pers = ctx.enter_context(tc.tile_pool(name="pers", bufs=1))
        khl = [pers.tile([128, T], BF16, name=f"khl{h}") for h in range(NH)]
        kha = [pers.tile([65, T], BF16, name=f"kha{h}") for h in range(NH)]
        qhh = [pers.tile([128, T], BF16, name=f"qhh{h}") for h in range(NH)]
        qla = [pers.tile([65, T], BF16, name=f"qla{h}") for h in range(NH)]
        v16 = [pers.tile([128, NH * VW], BF16, name=f"v16_{i}") for i in range(TC)]
        ident = pers.tile([128, 128], BF16, name="ident")
        nc.sync.dma_start(out=ident, in_=ident_d[:, :])
        ctxT_t = [pers.tile([128, T], BF16, name=f"ctxT{c}") for c in range(2)]

        # ---------------- phase 1: projections ----------------
        with ExitStack() as ph1:
            ld = ph1.enter_context(tc.tile_pool(name="ld", bufs=1))
            wpool = ph1.enter_context(tc.tile_pool(name="wpool", bufs=2))
            pps = ph1.enter_context(tc.tile_pool(name="pps", bufs=2, space="PSUM"))
            upsp = ph1.enter_context(tc.tile_pool(name="upsp", bufs=1, space="PSUM"))
            vtrp = ph1.enter_context(tc.tile_pool(name="vtrp", bufs=2, space="PSUM"))

            for h in range(NH):
                nc.vector.memset(kha[h][64:65, :], 1.0)

            xth_t, xtl_t, ah_t, al_t = [], [], [], []
            for ci in range(CI):
                t_ = ld.tile([128, T], BF16, name=f"xth{ci}")
                nc.sync.dma_start(out=t_, in_=xth_d[ts(ci, 128), :])
                xth_t.append(t_)
                t_ = ld.tile([128, T], BF16, name=f"xtl{ci}")
                nc.sync.dma_start(out=t_, in_=xtl_d[ts(ci, 128), :])
                xtl_t.append(t_)
                if lora:
                    t_ = ld.tile([128, 3 * R], BF16, name=f"ah{ci}")
                    nc.sync.dma_start(out=t_, in_=ah_d[ts(ci, 128), :])
                    ah_t.append(t_)
                    t_ = ld.tile([128, 3 * R], BF16, name=f"al{ci}")
                    nc.sync.dma_start(out=t_, in_=al_d[ts(ci, 128), :])
                    al_t.append(t_)
            u_bf = {}
            b_t = {}
            if lora:
                for key2, d in b_d.items():
                    t_ = ld.tile([R, OL], BF16, name=f"b{key2}")
                    nc.sync.dma_start(out=t_, in_=d[:, :])
                    b_t[key2] = t_

                # u_all = x @ A_all (split3), shared M=24 pass
                ups = upsp.tile([3 * R, T], F32, name="ups")
                for ns in range(NS):
                    sl = ts(ns, 512)
                    n_mm = 3 * CI
                    i = 0
                    for ci in range(CI):
                        for a_t, x_t in ((ah_t[ci], xth_t[ci]), (ah_t[ci], xtl_t[ci]),
                                         (al_t[ci], xth_t[ci])):
                            nc.tensor.matmul(ups[:, sl], a_t, x_t[:, sl],
                                             start=(i == 0), stop=(i == n_mm - 1))
                            i += 1
                uf = ld.tile([3 * R, T], F32, name="uf")
                nc.any.tensor_copy(uf, ups)
                for pi, p in enumerate("qkv"):
                    upf = ld.tile([R, T], F32, tag="upf", name=f"u{p}f")
                    nc.sync.dma_start(out=upf, in_=uf[pi * R:(pi + 1) * R, :])
                    uh = ld.tile([R, T], BF16, name=f"u{p}h")
                    ul = ld.tile([R, T], BF16, name=f"u{p}l")
                    nc.vector.tensor_copy(uh, upf)
                    nc.vector.tensor_sub(ul, upf, uh)
                    u_bf[p + "h"], u_bf[p + "l"] = uh, ul

            # --- projections, transposed layout [OL, T] ---
            for p in "qkv":
                wh_t, wl_t = [], []
                for ci in range(CI):
                    t_ = wpool.tile([128, OL], BF16, tag=f"wh{ci}", name=f"wh{ci}")
                    nc.sync.dma_start(out=t_, in_=w_d[p + "h"][ts(ci, 128), :])
                    wh_t.append(t_)
                    if p != "v":
                        t_ = wpool.tile([128, OL], BF16, tag=f"wl{ci}", name=f"wl{ci}")
                        nc.sync.dma_start(out=t_, in_=w_d[p + "l"][ts(ci, 128), :])
                        wl_t.append(t_)
                vth_t = None
                if p == "v":
                    vth_t = [wpool.tile([128, T], BF16, tag=f"vth{c}", name=f"vth{c}",
                                        bufs=1) for c in range(2)]
                for oc in range(2):
                    osl = ts(oc, 128)
                    h0, h1 = 2 * oc, 2 * oc + 1
                    for ns in range(NS):
                        sl = ts(ns, 512)
                        ps = pps.tile([128, 512], F32, tag="proj", name="proj")
                        if p == "v":
                            base = [(wh_t[ci], xth_t[ci]) for ci in range(CI)]
                        else:
                            base = []
                            for ci in range(CI):
                                base += [(wh_t[ci], xth_t[ci]), (wh_t[ci], xtl_t[ci]),
                                         (wl_t[ci], xth_t[ci])]
                        seq = [(a[:, osl], b_[:, sl]) for a, b_ in base]
                        if lora:
                            seq += [(b_t[p + "h"][:, osl], u_bf[p + "h"][:, sl]),
                                    (b_t[p + "h"][:, osl], u_bf[p + "l"][:, sl]),
                                    (b_t[p + "l"][:, osl], u_bf[p + "h"][:, sl])]
                        for i, (a, b_) in enumerate(seq):
                            nc.tensor.matmul(ps, a, b_, start=(i == 0),
                                             stop=(i == len(seq) - 1))
                        if p == "v":
                            nc.any.tensor_copy(vth_t[oc][:, sl], ps)
                        elif p == "q":
                            for h, rows in ((h0, ps[0:64, :]), (h1, ps[64:128, :])):
                                nc.any.tensor_copy(qhh[h][0:64, sl], rows)
                                nc.any.tensor_copy(qhh[h][64:128, sl], rows)
                                nc.vector.tensor_sub(qla[h][0:64, sl], rows,
                                                     qhh[h][0:64, sl])
                        else:
                            for h, rows in ((h0, ps[0:64, :]), (h1, ps[64:128, :])):
                                nc.any.tensor_copy(khl[h][0:64, sl], rows)
                                nc.any.tensor_copy(kha[h][0:64, sl], rows)
                                nc.vector.tensor_sub(khl[h][64:128, sl], rows,
                                                     khl[h][0:64, sl])
                if p == "v":
                    # v16 tiles: per head 64 v-cols + a ones column (Z trick)
                    for tci in range(TC):
                        nc.vector.memset(v16[tci], 1.0)
                    for oc in range(2):
                        for tci in range(TC):
                            tp = vtrp.tile([128, 128], BF16, tag="vtr", name="vtr")
                            nc.tensor.transpose(tp, vth_t[oc][:, ts(tci, 128)], ident)
                            h0, h1 = 2 * oc, 2 * oc + 1
                            nc.any.tensor_copy(v16[tci][:, h0 * VW:h0 * VW + 64],
                                               tp[:, 0:64])
                            nc.any.tensor_copy(v16[tci][:, h1 * VW:h1 * VW + 64],
                                               tp[:, 64:128])

        # ---------------- phase 3: attention ----------------
        with ExitStack() as ph3:
            att = ph3.enter_context(tc.tile_pool(name="att", bufs=2))
            ptp = ph3.enter_context(tc.tile_pool(name="ptp", bufs=2))
            drp = ph3.enter_context(tc.tile_pool(name="drp", bufs=2, space="DRAM"))
            msp = ph3.enter_context(tc.tile_pool(name="msp", bufs=1, space="PSUM"))
            sps = ph3.enter_context(tc.tile_pool(name="sps", bufs=4, space="PSUM"))
            cps = ph3.enter_context(tc.tile_pool(name="cps", bufs=1, space="PSUM"))
            mtp = ph3.enter_context(tc.tile_pool(name="mtp", bufs=1, space="PSUM"))

            def mhat_pass(h):
                # m-hat for head h: [q,k] single-bf16 scores -> row max ->
                # -m-hat lands in qla[h] row 64 via a DRAM transpose bounce, so
                # the K=65 score matmul subtracts it inside PSUM for free
                rm16a = att.tile([128, 16], F32, name="rm16a")
                rm16b = att.tile([128, 16], F32, name="rm16b")
                for qt in range(TC):
                    for half, rm in ((0, rm16a), (1, rm16b)):
                        ms = msp.tile([128, T // 2], F32, tag="ms", name="ms")
                        for nsj in range(2):
                            ns = half * 2 + nsj
                            nc.tensor.matmul(ms[:, ts(nsj, 512)],
                                             qhh[h][0:64, ts(qt, 128)],
                                             khl[h][0:64, ts(ns, 512)],
                                             start=True, stop=True)
                        nc.vector.reduce_max(out=rm[:, qt:qt + 1], in_=ms, axis=AX.X)
                rm16 = att.tile([128, 16], F32, name="rm16")
                nc.vector.tensor_max(rm16, rm16a, rm16b)
                rm16s = att.tile([128, 16], BF16, name="rm16s")
                nc.vector.tensor_scalar_mul(rm16s, rm16, -1.0)
                # transpose on PE, then a burst-contiguous DRAM bounce:
                # qla[h][64, qt*128+q] = rm16s[q, qt]
                mtr = mtp.tile([16, 128], BF16, tag="mtr", name="mtr")
                nc.tensor.transpose(mtr, rm16s, ident)
                rmT = att.tile([16, 128], BF16, name="rmT")
                nc.any.tensor_copy(rmT, mtr)
                dr = drp.tile([16, 128], BF16, name="mh_dr")
                nc.sync.dma_start(out=dr, in_=rmT)
                src = bass.AP(tensor=dr.tensor, offset=dr.offset,
                              ap=[[1, 16 * 128]])
                nc.sync.dma_start(out=qla[h][64:65, :], in_=src)

            mhat_pass(0)
            for h in range(NH):
                ch = h // 2
                pr = (h % 2) * 64
                if h + 1 < NH:
                    mhat_pass(h + 1)   # pipelined one head ahead

                for qb in range(NS):
                    qsl = ts(qb, 512)
                    # --- sT pass: K-stacked scores with fused -m-hat -> exp ---
                    pT = [ptp.tile([128, 512], BF16, tag=f"pt{i}", name=f"pt{i}")
                          for i in range(TC)]
                    for kt in range(TC):
                        st = sps.tile([128, 512], F32, tag="st", name="st")
                        # kh·qh + kl·qh in one K=128 matmul (qh duplicated)
                        nc.tensor.matmul(st, khl[h][:, ts(kt, 128)], qhh[h][:, qsl],
                                         start=True, stop=False)
                        # kh·ql + ones·(-m-hat), K=65
                        nc.tensor.matmul(st, kha[h][:, ts(kt, 128)],
                                         qla[h][:, qsl], start=False, stop=True)
                        nc.scalar.activation(out=pT[kt], in_=st, func=Exp, scale=0.125)
                    # --- PV with ones column ---
                    cxa = cps.tile([VW, 512], F32, tag="cxa", name="cxa")
                    for kt in range(TC):
                        nc.tensor.matmul(cxa, v16[kt][:, h * VW:(h + 1) * VW], pT[kt],
                                         start=(kt == 0), stop=(kt == TC - 1))
                    # --- normalize by Z (row 64) during evacuation ---
                    zrow = att.tile([1, 512], F32, name="zrow")
                    nc.vector.tensor_copy(zrow, cxa[64:65, :])
                    z_bc = att.tile([64, 512], F32, name="z_bc")
                    nc.gpsimd.partition_broadcast(z_bc, zrow, channels=64)
                    rcp_bc = att.tile([64, 512], F32, name="rcp_bc")
                    nc.vector.reciprocal_approx_fast(out=rcp_bc, in_=z_bc)
                    nc.vector.tensor_mul(ctxT_t[ch][pr:pr + 64, qsl], cxa[0:64, :],
                                         rcp_bc)

        # ---------------- phase 4: output projection ----------------
        with ExitStack() as ph4:
            ost_p = ph4.enter_context(tc.tile_pool(name="ost", bufs=3))
            ops = ph4.enter_context(tc.tile_pool(name="ops", bufs=2, space="PSUM"))
            woT_t = []
            for cc in range(2):
                t_ = ost_p.tile([128, E], BF16, tag=f"woT{cc}", name=f"woT{cc}")
                nc.sync.dma_start(out=t_, in_=woT_d[ts(cc, 128), :])
                woT_t.append(t_)
            for tci in range(TC):
                tsl = ts(tci, 128)
                ost = ost_p.tile([128, E], F32, tag="ost", name="ost")
                for no in range(2):
                    op_t = ops.tile([128, 512], F32, tag=f"op{no}", name=f"op{no}")
                    for cc in range(2):
                        nc.tensor.matmul(op_t, ctxT_t[cc][:, tsl],
                                         woT_t[cc][:, ts(no, 512)],
                                         start=(cc == 0), stop=(cc == 1))
                    nc.any.tensor_copy(ost[:, ts(no, 512)], op_t)
                nc.sync.dma_start(out=outp_d[tsl, :], in_=ost)

    nc.compile()
    _CACHE[key] = nc
    return nc


def _split(a):
    h = a.astype(bf16)
    l = (a - h.astype(np.float32)).astype(bf16)
    return h, l


def _shard(inputs, lora):
    x = np.asarray(inputs["x"], np.float32)
    Wo = np.asarray(inputs["Wo"], np.float32)
    ident = np.eye(128, dtype=np.float32).astype(bf16)
    if lora:
        A_all = np.concatenate([np.asarray(inputs["Aq"], np.float32),
                                np.asarray(inputs["Ak"], np.float32),
                                np.asarray(inputs["Av"], np.float32)], axis=1)
        ah, al = _split(A_all)
    in_maps = []
    for core in range(8):
        b, hp = core // 4, core % 4
        o0 = hp * OL
        xT = np.ascontiguousarray(x[b].T)
        xh, xl = _split(xT)
        m = {"xth": xh, "xtl": xl, "ident": ident}
        for p in "qkv":
            W = np.asarray(inputs["W" + p], np.float32)
            Ws = np.ascontiguousarray(W[o0:o0 + OL, :].T)
            m["w%sh" % p], m["w%sl" % p] = _split(Ws)
            if lora:
                B = np.asarray(inputs["B" + p], np.float32)[:, o0:o0 + OL] * 2.0
                m["b%sh" % p], m["b%sl" % p] = _split(B)
        m["woT"] = np.ascontiguousarray(Wo[:, o0:o0 + OL].T).astype(bf16)
        if lora:
            m["ah"], m["al"] = ah, al
        in_maps.append(m)
    return in_maps


def _run(inputs, trace=False, **kw):
    lora = not all(
        np.count_nonzero(np.asarray(inputs["B" + p])) == 0 for p in "qkv")
    nc = _build(lora)
    in_maps = _shard(inputs, lora)
    res = run_bass_kernel_spmd(nc, in_maps, core_ids=list(range(8)), trace=trace, **kw)
    bo = np.asarray(inputs["bo"], np.float32)
    parts = [res.results[c]["outp"].astype(np.float64) for c in range(8)]
    out = np.stack([sum(parts[0:4]), sum(parts[4:8])]) + bo.astype(np.float64)
    return out.astype(np.float32), res


def kernel(**inputs):
    out, _ = _run(inputs)
    return out
